# revision 1
# baseline (speedup 1.0000x reference)
import os

import numpy as np

B, IN, OUT = 8192, 4096, 4096
NCORES = 8
PB, PO = 4, 2
MB = B // PB
NO = OUT // PO
P = 128
KT = IN // P
NF = 512
C = 256
NCH = MB // C
MT = MB // P
NW = NO // NF
KL = 28

_CACHE = {}


def _build_nc(reps=1):
    import concourse.mybir as mybir
    from concourse import bacc, tile

    f32 = mybir.dt.float32
    bf16 = mybir.dt.bfloat16
    fp8 = mybir.dt.float8e4
    DR = mybir.MatmulPerfMode.DoubleRow
    Alu = mybir.AluOpType
    Act = mybir.ActivationFunctionType

    nc = bacc.Bacc("TRN2", target_bir_lowering=False, debug=False)

    xT_d = nc.dram_tensor("xT", [IN, MB], f32, kind="ExternalInput")
    wT_d = nc.dram_tensor("WT", [IN, NO], f32, kind="ExternalInput")
    b_d = nc.dram_tensor("b", [1, NO], f32, kind="ExternalInput")
    o_d = nc.dram_tensor("out", [MB, NO], bf16, kind="ExternalOutput")

    with tile.TileContext(nc) as tc:
        with (
            tc.tile_pool(name="const", bufs=1) as const,
            tc.tile_pool(name="w8w", bufs=2) as w8_pool,
            tc.tile_pool(name="xh", bufs=3) as xh_pool,
            tc.tile_pool(name="xl", bufs=1) as xl_pool,
            tc.tile_pool(name="tmp", bufs=2) as tmp_pool,
            tc.tile_pool(name="x2", bufs=2) as x2_pool,
            tc.tile_pool(name="stats", bufs=2) as stats,
            tc.tile_pool(name="osb", bufs=3) as osb_pool,
            tc.tile_pool(name="psum", bufs=6, space="PSUM") as psum_pool,
            tc.tile_pool(name="npsum", bufs=2, space="PSUM") as npsum_pool,
            tc.tile_pool(name="dram", bufs=1, space="DRAM") as dram_pool,
        ):
            bias_sb = const.tile([1, NO], bf16)
            nc.gpsimd.dma_start(out=bias_sb[:], in_=b_d[:])
            ones8 = const.tile([P, 2, P], fp8)
            nc.any.memset(ones8[:], 1.0)
            zero_bias = const.tile([P, 1], f32)
            nc.any.memset(zero_bias[:], 0.0)

            for _rep in range(reps):
                xls = [
                    xl_pool.tile([P, KL, C], fp8, name=f"xl_{_rep}_{c}")
                    for c in range(NCH)
                ]
                nrm16s = [
                    stats.tile([1, C], bf16, name=f"nrm16_{_rep}_{c}", bufs=1)
                    for c in range(NCH)
                ]
                rcp64s = [
                    stats.tile([P, 2], f32, name=f"rcp64_{_rep}_{c}", bufs=1)
                    for c in range(NCH)
                ]
                nrm_d = dram_pool.tile([NCH, C], f32, name=f"nrm_d_{_rep}")
                w8s, xhs = {}, {}

                def x_cols(c, w=C):
                    return xT_d[:, c * w : (c + 1) * w].rearrange(
                        "(kt p) m -> p kt m", p=P
                    )

                def load_xh(cc):
                    xh = xh_pool.tile([P, KT, 2 * C], fp8, tag="xh")
                    nc.gpsimd.dma_start(out=xh[:], in_=x_cols(cc, 2 * C))
                    xhs[cc] = xh

                def xh_slice(c):
                    return xhs[c // 2][:, :, (c % 2) * C : (c % 2 + 1) * C]

                def build_w_win(n, pieces=2, only_h=None):
                    if only_h is None or only_h == 0:
                        w8 = w8_pool.tile([P, KT, NF], fp8, tag="w8w")
                        w8s[n] = w8
                    w8 = w8s[n]
                    for h in ((0, 1) if only_h is None else (only_h,)):
                        wc = tmp_pool.tile([P, KT, C], bf16, tag="tmp")
                        src = wT_d[:, n * NF + h * C : n * NF + (h + 1) * C]
                        nc.gpsimd.dma_start(
                            out=wc[:], in_=src.rearrange("(kt p) n -> p kt n", p=P)
                        )
                        w = C // pieces
                        for q in range(pieces):
                            dst = w8[:, :, h * C + q * w : h * C + (q + 1) * w]
                            on_act = (h * pieces + q) % 2 == 0
                            if on_act:
                                nc.scalar.mul(dst, wc[:, :, q * w : (q + 1) * w], 64.0)
                            else:
                                nc.vector.tensor_scalar(
                                    out=dst, in0=wc[:, :, q * w : (q + 1) * w],
                                    scalar1=64.0, scalar2=0.0,
                                    op0=Alu.mult, op1=Alu.add,
                                )

                def chunk_sumsq(c):
                    x2 = x2_pool.tile([P, KT, C], fp8, tag="x2")
                    nc.scalar.square(x2[:], xh_slice(c))
                    ps_n = npsum_pool.tile([P, C], f32, tag="nps")
                    for kt in range(KT // 2):
                        nc.tensor.matmul(
                            ps_n[:], lhsT=ones8[:],
                            rhs=x2[:, 2 * kt : 2 * kt + 2, :],
                            start=(kt == 0), stop=(kt == KT // 2 - 1),
                            perf_mode=DR,
                        )
                    nrmf = stats.tile([1, C], f32, tag="nrmf")
                    nc.scalar.activation(nrmf[:], ps_n[0:1, :], Act.Sqrt, scale=4096.0)
                    nc.vector.tensor_copy(nrm16s[c][:], nrmf[:])
                    rcpf = stats.tile([1, C], f32, tag="rcpf")
                    nc.vector.reciprocal(rcpf[:], nrmf[:])
                    nc.sync.dma_start(out=nrm_d[c : c + 1, :], in_=rcpf[:])
                    nc.sync.dma_start(
                        out=rcp64s[c][:],
                        in_=nrm_d[c : c + 1, :].rearrange(
                            "o (j p) -> (o p) j", p=P
                        ),
                    )

                def chunk_xlo(c):
                    xc = tmp_pool.tile([P, KL, C], bf16, tag="tmp")
                    src = xT_d[: KL * P, c * C : (c + 1) * C].rearrange(
                        "(kt p) m -> p kt m", p=P
                    )
                    nc.gpsimd.dma_start(out=xc[:], in_=src)
                    xl = xls[c]
                    nc.vector.scalar_tensor_tensor(
                        out=xl[:], in0=xc[:], scalar=1.0,
                        in1=xh_slice(c)[:, :KL, :],
                        op0=Alu.mult, op1=Alu.subtract,
                    )

                def build_x_chunk(c):
                    chunk_sumsq(c)
                    chunk_xlo(c)

                def group_mms(n, mb):
                    c, j = mb // 2, mb % 2
                    hi = xhs[mb // 4][:, :, (mb % 4) * P : (mb % 4 + 1) * P]
                    lo = xls[c][:, :, j * P : (j + 1) * P]
                    ps = psum_pool.tile([P, NF], f32, tag="acc")
                    for src_i, (src, nk) in enumerate(((hi, KT), (lo, KL))):
                        for kk in range(nk // 2):
                            nc.tensor.matmul(
                                ps[:],
                                lhsT=src[:, 2 * kk : 2 * kk + 2, :],
                                rhs=w8s[n][:, 2 * kk : 2 * kk + 2, :],
                                start=(src_i == 0 and kk == 0),
                                stop=False,
                                perf_mode=DR,
                            )
                    return ps

                def group_close(n, mb, ps):
                    c, j = mb // 2, mb % 2
                    nc.tensor.matmul(
                        ps[:],
                        lhsT=nrm16s[c][:, j * P : (j + 1) * P],
                        rhs=bias_sb[:, n * NF : (n + 1) * NF],
                        start=False,
                        stop=True,
                    )
                    osb = osb_pool.tile([P, NF], bf16, tag="osb")
                    scale = rcp64s[c][:, j : j + 1]
                    if mb % 2 == 0:
                        nc.scalar.activation(
                            osb[:], ps[:], Act.Relu, bias=zero_bias[:], scale=scale
                        )
                    else:
                        nc.vector.tensor_scalar(
                            out=osb[:], in0=ps[:], scalar1=scale, scalar2=0.0,
                            op0=Alu.mult, op1=Alu.max,
                        )
                    nc.sync.dma_start(
                        out=o_d[mb * P : (mb + 1) * P, n * NF : (n + 1) * NF],
                        in_=osb[:],
                    )

                def group(n, mb):
                    group_close(n, mb, group_mms(n, mb))

                load_xh(0)
                build_w_win(0, pieces=4)
                chunk_xlo(0)
                ps00 = group_mms(0, 0)
                chunk_sumsq(0)
                group_close(0, 0, ps00)

                passes = [(0, 0), (1, 0), (0, 1), (1, 1),
                          (2, 0), (3, 0), (2, 1), (3, 1)]
                xh_loads = {
                    (0, 1): 1, (1, 0): 2, (1, 4): 3,
                    (3, 0): 0, (3, 4): 1, (4, 0): 2, (5, 4): 3,
                }
                builds = {
                    (0, 1): 1, (0, 3): 2, (0, 5): 3,
                    (1, 1): 4, (1, 3): 5, (1, 5): 6, (2, 1): 7,
                }
                w_builds = {(0, 2): (1, 0), (0, 5): (1, 1),
                            (3, 0): (2, None), (4, 0): (3, None)}
                for p, (n, half) in enumerate(passes):
                    for q in range(8):
                        mb = half * 8 + q
                        if p == 0 and q == 0:
                            continue
                        if (p, q) in xh_loads:
                            load_xh(xh_loads[(p, q)])
                        if (p, q) in w_builds:
                            wn, wh = w_builds[(p, q)]
                            build_w_win(wn, only_h=wh)
                        if (p, q) in builds:
                            build_x_chunk(builds[(p, q)])
                        group(n, mb)

    nc.compile()
    return nc


def _get_nc():
    if "nc" not in _CACHE:
        os.environ.setdefault("MYCRO_LOCAL_CACHE", "1")
        _CACHE["nc"] = _build_nc()
    return _CACHE["nc"]


def _make_in_maps(x, W, b):
    xT = np.ascontiguousarray(np.asarray(x, dtype=np.float32).T)
    WT = np.ascontiguousarray(np.asarray(W, dtype=np.float32).T)
    b = np.asarray(b, dtype=np.float32).reshape(-1)
    in_maps = []
    for i in range(NCORES):
        ib, io = i // PO, i % PO
        in_maps.append({
            "xT": np.ascontiguousarray(xT[:, ib * MB : (ib + 1) * MB]),
            "WT": np.ascontiguousarray(WT[:, io * NO : (io + 1) * NO]),
            "b": np.ascontiguousarray(b[io * NO : (io + 1) * NO]).reshape(1, NO),
        })
    return in_maps


def kernel(x, W, b):
    from concourse.bass_utils import run_bass_kernel_spmd

    x = np.asarray(x, dtype=np.float32)
    W = np.asarray(W, dtype=np.float32)
    assert x.shape == (B, IN) and W.shape == (OUT, IN)

    nc = _get_nc()
    res = run_bass_kernel_spmd(nc, _make_in_maps(x, W, b),
                               core_ids=list(range(NCORES)))
    out = np.empty((B, OUT), dtype=np.float32)
    for i in range(NCORES):
        ib, io = i // PO, i % PO
        out[ib * MB : (ib + 1) * MB, io * NO : (io + 1) * NO] = np.asarray(
            res.results[i]["out"]
        ).astype(np.float32)
    return out

